# revision 17
# baseline (speedup 1.0000x reference)
"""RNN-T Joiner kernel for Trainium2 (Bass/Tile), 8-core data-parallel over batch.

out[b,t,u,v] = (enc[b,t] @ We)[v] + (pred[b,u] @ Wp)[v] + bias[v]

Layout trick: put V on SBUF partitions. Then for a fixed u, the pred term is a
per-partition scalar, so the broadcast-add is a DVE tensor_scalar_add (2x_1p
fp16 mode) or an Act-engine activation with per-partition bias — no PE one-hot
broadcast and no fp32 tensor_tensor adds. The fp16 datapath halves HBM store
traffic (34 MB/core vs 68 MB), which is the roofline term. Tolerance is 2e-2
rel; fp16 rounding contributes ~6e-4.

The vocab is processed in two halves (vc 0-3, vc 4-7): half A's projections,
evacuations, adds and stores all start while the PE (in-order) is still
projecting half B, hiding most of the projection phase behind the store
stream. Accumulation groups are per-vc contiguous in their own PSUM slots
(start=True clears has_written for the whole bank, so groups must not
interleave within a bank).

Per core (one batch element):
  - Loads: host-pretiled, enc-side on the Sync HWDGE ring, pred-side on the
    Act ring, in dependency order.
  - PE (fp16): enc_projT [v,t] / pred_projT [v,u] with V on output
    partitions; bias folded in via a ones-row matmul.
  - Act: PSUM->SBUF evacuation + ~4/13 of the per-u adds.
  - DVE: ~9/13 of the per-u adds.
  - HWDGE DMA: stores with HBM layout [v_lo, v_half, u, vc4, t] (~14-18 KB
    contiguous runs), u-blocks 15,15,15,15,5 stored per (half, u-split);
    block 0 leads with 3-u stores, the small last block shortens the tail.
    Un-permuted on the host.
"""

import sys

sys.path.insert(0, "/opt/trn_rl_repo")

import numpy as np

B, T, U1, D, V = 8, 256, 65, 640, 1024
KC = 5          # contraction chunks (D = 5*128)
VC = 8          # vocab chunks (V = 8*128)
VH = VC // 2    # vc per half
UBLOCKS = (15, 15, 15, 15, 5)
NACT_RATIO = (9, 13)  # ops with idx%13 >= 9 go to Act

_COMPILED = None


def _build():
    import concourse.bacc as bacc
    import concourse.tile as tile
    import concourse.mybir as mybir

    f16 = mybir.dt.float16
    f32 = mybir.dt.float32

    nc = bacc.Bacc("TRN2", target_bir_lowering=False, debug=False, num_devices=8)

    # host-pretiled: encT[p, c, t] = enc.T[c*128+p, t], etc.
    encT = nc.dram_tensor("encT", [128, KC * T], f16, kind="ExternalInput")
    predT = nc.dram_tensor("predT", [128, KC * U1], f16, kind="ExternalInput")
    We = nc.dram_tensor("We", [128, KC * V], f16, kind="ExternalInput")
    Wp = nc.dram_tensor("Wp", [128, KC * V], f16, kind="ExternalInput")
    bias = nc.dram_tensor("bias", [1, V], f16, kind="ExternalInput")
    ones = nc.dram_tensor("ones", [1, U1], f16, kind="ExternalInput")
    # out[v_lo, v_half, u, vc_in_half, t] ; v = (vh*VH + vch)*128 + v_lo
    out = nc.dram_tensor("out", [128, 2, U1, VH, T], f16, kind="ExternalOutput")

    with tile.TileContext(nc) as tc:
        with tc.tile_pool(name="consts", bufs=1) as cp:
            encT_sb = cp.tile([128, KC * T], f16, tag="encT")
            nc.sync.dma_start(encT_sb[:], encT[:])
            We_sb = cp.tile([128, KC * V], f16, tag="We")
            nc.sync.dma_start(We_sb[:], We[:])
            predT_sb = cp.tile([128, KC * U1], f16, tag="predT")
            nc.scalar.dma_start(predT_sb[:], predT[:])
            Wp_sb = cp.tile([128, KC * V], f16, tag="Wp")
            nc.scalar.dma_start(Wp_sb[:], Wp[:])
            bias_sb = cp.tile([1, V], f16, tag="bias")
            nc.scalar.dma_start(bias_sb[:], bias[:])
            ones_sb = cp.tile([1, U1], f16, tag="ones")
            nc.scalar.dma_start(ones_sb[:], ones[:])

            encP = cp.tile([128, VC * T], f16, tag="encP")      # enc_projT[v, t]
            predP = cp.tile([128, VC * U1], f32, tag="predP")   # pred_projT[v, u] + b[v]

            # ---- projections per vc-half: A fully, then B ----
            with tc.tile_pool(name="ppool", bufs=2, space="PSUM") as pp:
                for half in (range(0, VH), range(VH, VC)):
                    for vc in half:
                        vs = slice(vc * 128, (vc + 1) * 128)
                        pse = pp.tile([128, T], f32, tag="pse")
                        for c in range(KC):
                            nc.tensor.matmul(
                                pse[:],
                                We_sb[:, c * V + vc * 128:c * V + (vc + 1) * 128],
                                encT_sb[:, c * T:(c + 1) * T],
                                start=(c == 0), stop=(c == KC - 1))
                        nc.scalar.copy(encP[:, vc * T:(vc + 1) * T], pse[:])
                    for vc in half:
                        psp = pp.tile([128, U1], f32, tag="psp")
                        for c in range(KC):
                            nc.tensor.matmul(
                                psp[:],
                                Wp_sb[:, c * V + vc * 128:c * V + (vc + 1) * 128],
                                predT_sb[:, c * U1:(c + 1) * U1],
                                start=(c == 0), stop=False)
                        nc.tensor.matmul(
                            psp[:], bias_sb[0:1, vc * 128:(vc + 1) * 128],
                            ones_sb[0:1, :], start=False, stop=True)
                        nc.scalar.copy(predP[:, vc * U1:(vc + 1) * U1], psp[:])

            # ---- main loop: per-u scalar-add, stores per (half, u-split) ----
            opidx = 0
            with tc.tile_pool(name="outp", bufs=2) as op_:
                u0 = 0
                for blk, nu in enumerate(UBLOCKS):
                    stage = op_.tile(
                        [128, 2, max(UBLOCKS), VH, T], f16, tag="stage")
                    if blk == 0:
                        splits = ((0, 3), (3, 9), (9, 15))
                    elif nu == 15:
                        splits = ((0, 8), (8, 15))
                    else:
                        splits = ((0, 3), (3, nu))
                    for vh in range(2):
                        for lo, hi in splits:
                            for ui in range(lo, hi):
                                u = u0 + ui
                                for vch in range(VH):
                                    vc = vh * VH + vch
                                    enc_ap = encP[:, vc * T:(vc + 1) * T]
                                    sc_ap = predP[
                                        :, vc * U1 + u:vc * U1 + u + 1]
                                    if opidx % NACT_RATIO[1] >= NACT_RATIO[0]:
                                        nc.scalar.add(
                                            stage[:, vh, ui, vch, :],
                                            enc_ap, sc_ap)
                                    else:
                                        nc.vector.tensor_scalar_add(
                                            stage[:, vh, ui, vch, :],
                                            enc_ap, sc_ap)
                                    opidx += 1
                            nc.sync.dma_start(
                                out[:, vh, u0 + lo:u0 + hi, :, :],
                                stage[:, vh, lo:hi, :, :])
                    u0 += nu

    nc.compile()
    return nc


def _get_compiled():
    global _COMPILED
    if _COMPILED is None:
        _COMPILED = _build()
    return _COMPILED


def _in_maps(encoder_out, predictor_out, W, b):
    Wt = np.asarray(W, dtype=np.float16).reshape(2 * KC, 128, V)
    We = np.ascontiguousarray(Wt[:KC].transpose(1, 0, 2)).reshape(128, KC * V)
    Wp = np.ascontiguousarray(Wt[KC:].transpose(1, 0, 2)).reshape(128, KC * V)
    bias = np.ascontiguousarray(np.asarray(b, dtype=np.float16).reshape(1, V))
    ones = np.ones((1, U1), dtype=np.float16)
    maps = []
    for i in range(B):
        et = np.asarray(encoder_out[i], dtype=np.float16).T  # [D, T]
        pt = np.asarray(predictor_out[i], dtype=np.float16).T  # [D, U1]
        maps.append({
            "encT": np.ascontiguousarray(
                et.reshape(KC, 128, T).transpose(1, 0, 2)).reshape(128, KC * T),
            "predT": np.ascontiguousarray(
                pt.reshape(KC, 128, U1).transpose(1, 0, 2)).reshape(128, KC * U1),
            "We": We,
            "Wp": Wp,
            "bias": bias,
            "ones": ones,
        })
    return maps


def run(encoder_out, predictor_out, W, b, trace=False, tmpdir=None):
    from concourse.bass_utils import run_bass_kernel_spmd

    nc = _get_compiled()
    maps = _in_maps(encoder_out, predictor_out, W, b)
    res = run_bass_kernel_spmd(
        nc, maps, list(range(B)), trace=trace,
        **({"tmpdir": tmpdir} if tmpdir else {}))
    outs = np.empty((B, T, U1, V), dtype=np.float32)
    for i in range(B):
        arr = res.results[i]["out"]  # [128, 2, U1, VH, T] fp16
        outs[i] = arr.transpose(4, 2, 1, 3, 0).reshape(T, U1, V).astype(np.float32)
    return outs, res


def kernel(encoder_out, predictor_out, W, b):
    outs, _ = run(encoder_out, predictor_out, W, b)
    return outs
